# revision 76
# baseline (speedup 1.0000x reference)
"""MoE LoRA delta kernel for Trainium2 (Bass/Tile), 8-core SPMD.

Computation (reference):
  flat [T,F] -> logits = flat @ Wr.T [T,E]; top-2 softmax gates
  mid = flat @ A[e].T  [T,E,R];  delta = sum_e gates[:,e] * (mid[e] @ B[e].T) * SCALE

Shapes: T=4096 (2x2048), F=O=4096, E=4, R=16, SCALE=4.0.

Strategy (v2):
  - Token-shard T across 8 cores (512 tokens each); replicate tiny weights.
  - x is cast to bf16 and pre-permuted on host to xp[kb*TC+t, p] =
    x[t, kb*128+p]; the DMA XBAR transpose (dma_start(transpose=True))
    then lands x^T in SBUF as [128p, KB, TC] with fully-contiguous DRAM
    reads and zero PE/DVE transpose work.
  - Router Wr is folded into the first matmul: W1ext = [A_cat; Wr] ->
    [68, 4096]; one 32-step bf16 accumulation chain computes
    midT_ext [68, 512] (rows 0:64 = A-mid, rows 64:68 = router logits).
  - Gates: logits^T rows transposed per 128-token chunk via a tiny 4x4
    identity at partition offset 64; top-2 softmax via exp/masking on
    DVE/ACT; transpose back; replicate 4->64 rows via a 0/1 matmul.
  - mm2 per token chunk: delta [128t, 512o] = (gates*midT).T @ B_cat.T in
    fp32r (full PE rate at N=512); SCALE folded into B host-side.
  - Output staged per token chunk [128, 4096] and stored with 16KB/partition
    DMA lines.
"""

import numpy as np
import ml_dtypes

import concourse.bass as bass
import concourse.mybir as mybir
import concourse.tile as tile
from concourse import bacc, masks
from concourse.bass_utils import run_bass_kernel_spmd

F32 = mybir.dt.float32
F32R = mybir.dt.float32r
BF16 = mybir.dt.bfloat16

N_CORES = 8
T_FULL = 4096          # 2*2048 tokens
TC = T_FULL // N_CORES  # 512 tokens per core
F = 4096
O = 4096
E = 4
R = 16
ER = E * R             # 64
M1 = ER + E            # 68 (A rows + router rows)
KB = F // 128          # 32 f-blocks
NT = TC // 128         # 4 token chunks per core
NO = O // 512          # 8 output column chunks
NG = 4                 # x^T DMA-transpose groups
G = KB // NG           # k-blocks per group
SCALE = 16.0 / np.sqrt(16.0)  # 4.0


def _build_nc(dbg=False):
    nc = bacc.Bacc(
        "TRN2", debug=False, target_bir_lowering=False, enable_partition_id=False
    )

    # xp packed+transposed host-side: xp[kb*TC + t, p] = bf16(x)[t, kb*128+p]
    xp = nc.dram_tensor("xp", [KB * TC, 128], BF16, kind="ExternalInput")
    # w1 packed: [128, KB*M1 (+ER repm tail)]; w1[p, k*M1 + j] = W1ext[j, k*128 + p]
    # where W1ext = concat([A_cat (64 rows), Wr (4 rows)])  (bf16);
    # the trailing ER columns of rows 0:4 hold the 4->64 replication matrix
    w1 = nc.dram_tensor("w1", [128, KB * M1 + ER], BF16, kind="ExternalInput")
    # bt: [128, O]; rows 0:64 hold bt[e*R+r, o] = B[e, o, r] * SCALE, rows
    # 64:128 are zero padding: a hot bf16 K=128 matmul sustains a ~260ns
    # cadence (incl. weight reload) vs f32r-K=64's bandwidth-capped ~427ns
    bt = nc.dram_tensor("bt", [128, O], BF16, kind="ExternalInput")
    # bf16 output (host upconverts to f32): halves the output-DMA window,
    # which otherwise gates the entire mm2 phase
    out = nc.dram_tensor("out", [TC, O], BF16, kind="ExternalOutput")
    if dbg:
        dbg_mid = nc.dram_tensor("dbg_mid", [M1, TC], F32, kind="ExternalOutput")
        dbg_lg4 = nc.dram_tensor("dbg_lg4", [128, NT * E], F32, kind="ExternalOutput")
        dbg_g4 = nc.dram_tensor("dbg_g4", [128, NT * E], F32, kind="ExternalOutput")
        dbg_midp = nc.dram_tensor("dbg_midp", [ER, TC], F32R, kind="ExternalOutput")

    with tile.TileContext(nc) as tc:
        with (
            tc.tile_pool(name="consts", bufs=1) as consts,
            tc.tile_pool(name="xt", bufs=1) as xt_pool,
            tc.tile_pool(name="gates", bufs=2) as gp,
            tc.tile_pool(name="outp", bufs=4) as outp,
            tc.tile_pool(name="ps_mid", bufs=1, space="PSUM") as ps_mid,
            tc.tile_pool(name="ps_lg", bufs=1, space="PSUM") as ps_lg,
            tc.tile_pool(name="ps_g", bufs=1, space="PSUM") as ps_g,
            tc.tile_pool(name="ps_d", bufs=4, space="PSUM") as ps_d,
        ):
            # ---- constants / weights ----
            ident = consts.tile([128, 128], F32)
            masks.make_identity(nc, ident[:])
            # tiny 4x4 identity at partition offset 64 (for logit transposes:
            # lhsT/rhs must share a base partition, and logits live on 64:68)
            idq = consts.tile([128, E], F32)
            masks.make_identity(nc, idq[ER:M1, :])

            # ---- phase 1: XBAR-transpose x into SBUF, mm1 chain ----
            # The XBAR is a single serial unit and DMA traffic executes in
            # queue order, so the first transpose goes out immediately and the
            # small weight loads are interleaved between transposes. Group
            # sizes are uneven (10/10/10/2) so the last group's mm1 tail after
            # the final transpose is short.
            GS = [10, 10, 10, 2]
            GOFF = [0, 10, 20, 30]
            WARMS = [4, 4, 4]  # per-boundary warm counts
            # All DMA traffic is one serial pipe (the XBAR does not overlap
            # regular DMA), so order it by when each byte is first needed:
            # only the first group's w1 slice precedes XBAR g0; the rest of
            # the weights slot in between transposes. mm1 is PE-serial-bound,
            # and an earlier g0 directly advances its start.
            # Each XBAR<->regular-DMA mode transition stalls the sequencer
            # ~3us, so the XBAR transposes run as one consecutive block: w1
            # (needed by mm1 k=0) loads before it, bt (needed only by mm2,
            # ~30us later) after it.
            w1s = consts.tile([128, KB * M1 + ER], BF16)
            repm = w1s[0:E, KB * M1 : KB * M1 + ER]
            btb = consts.tile([128, O], BF16)
            nc.sync.dma_start(out=w1s[:], in_=w1[:])

            xts = []
            for g in range(NG):
                xT = xt_pool.tile([128, GS[g], TC], BF16, tag=f"xT{g}")
                nc.sync.dma_start(
                    out=xT[:],
                    in_=xp[GOFF[g] * TC : (GOFF[g] + GS[g]) * TC, :],
                    transpose=True,
                )
                xts.append(xT)

            nc.sync.dma_start(out=btb[:], in_=bt[:])

            def warm(n, g):
                # keep the PE's p-state hot across stalls: idle gaps drop it
                # to 1.2 GHz with a 3us re-ramp; a hot bf16 K=128 matmul
                # sustains ~260ns cadence vs ~430ns cold
                for _ in range(n):
                    pdw = ps_d.tile([128, 512], F32, tag="pd")
                    nc.tensor.matmul(
                        pdw[:], w1s[:, 0:128], xts[g][:, 0, :],
                        start=True, stop=True,
                    )

            midps = ps_mid.tile([M1, TC], F32)  # rows 0:64 mid, 64:68 logits
            for k in range(KB):
                g = next(i for i in range(NG) if GOFF[i] <= k < GOFF[i] + GS[i])
                nc.tensor.matmul(
                    midps[:],
                    w1s[:, k * M1 : (k + 1) * M1],
                    xts[g][:, k - GOFF[g], :],
                    start=(k == 0),
                    stop=(k == KB - 1),
                )
                if k == GOFF[g] + GS[g] - 1 and g < NG - 1:
                    warm(WARMS[g], g)  # bridge the wait for the next group
            warm(2, NG - 1)  # bridge the logits-eviction wait

            # ---- gates ----
            # logits^T [4, 512] live on partitions 64:68; evict lane-aligned
            lgT = gp.tile([128, TC], F32, tag="lgT")
            nc.vector.tensor_copy(out=lgT[ER:M1, :], in_=midps[ER:M1, :])
            # transpose to [t, e] layout per 128-token chunk
            lgps = ps_lg.tile([128, NT, E], F32)
            for t in range(NT):
                nc.tensor.matmul(
                    lgps[:, t, :],
                    lgT[ER:M1, t * 128 : (t + 1) * 128],
                    idq[ER:M1, :],
                    is_transpose=True,
                )
            # keep the PE continuously busy through the DVE softmax window:
            # an idle gap drops the PE to the 1.2 GHz p-state and re-ramping
            # to 2.4 GHz takes 3us of continuous execution, which would slow
            # every psgt/repmat/mm2 matmul afterwards to ~2x
            for i in range(10):
                pdw = ps_d.tile([128, 512], F32, tag="pd")
                nc.tensor.matmul(
                    pdw[:], w1s[:, 0:128], xts[NG - 1][:, 0, :],
                    start=True, stop=True,
                )

            lg4 = lgps  # softmax reads the PSUM logits directly
            if dbg:
                dmid = gp.tile([M1, TC], F32, tag="dmid")
                nc.vector.tensor_copy(out=dmid[:], in_=midps[:])
                nc.sync.dma_start(out=dbg_mid[:], in_=dmid[:])
                dlg = gp.tile([128, NT, E], F32, tag="dlg")
                nc.vector.tensor_copy(out=dlg[:], in_=lgps[:])
                nc.sync.dma_start(
                    out=dbg_lg4[:].rearrange("p (c e) -> p c e", e=E), in_=dlg[:]
                )

            # batched top-2 softmax over all chunks at once ([128, NT, E]);
            # no max-subtraction (logits are small, exp is safe in f32, and
            # exact equality with the running max identifies top-1/top-2)
            g4 = gp.tile([128, NT, E], F32, tag="g4")
            et = gp.tile([128, NT, E], F32, tag="et")
            nc.scalar.activation(et[:], lg4[:], mybir.ActivationFunctionType.Exp)
            mx4 = gp.tile([128, NT], F32, tag="mx4")
            nc.vector.reduce_max(mx4[:], et[:], axis=mybir.AxisListType.X)
            m1 = gp.tile([128, NT, E], F32, tag="m1")
            nc.vector.tensor_tensor(
                out=m1[:], in0=et[:],
                in1=mx4[:, :, None].broadcast_to([128, NT, E]),
                op=mybir.AluOpType.is_lt,
            )
            et2 = gp.tile([128, NT, E], F32, tag="et2")
            nc.vector.tensor_mul(et2[:], et[:], m1[:])  # et with top-1 zeroed
            mx2 = gp.tile([128, NT], F32, tag="mx2")
            nc.vector.reduce_max(mx2[:], et2[:], axis=mybir.AxisListType.X)
            m2 = gp.tile([128, NT, E], F32, tag="m2")
            nc.vector.tensor_tensor(
                out=m2[:], in0=et[:],
                in1=mx2[:, :, None].broadcast_to([128, NT, E]),
                op=mybir.AluOpType.is_ge,
            )
            em = gp.tile([128, NT, E], F32, tag="em")
            nc.vector.tensor_mul(em[:], et[:], m2[:])
            z = gp.tile([128, NT], F32, tag="z")
            nc.vector.reduce_sum(z[:], em[:], axis=mybir.AxisListType.X)
            rz = gp.tile([128, NT], F32, tag="rz")
            nc.vector.reciprocal(rz[:], z[:])
            nc.vector.tensor_mul(
                g4[:], em[:], rz[:, :, None].broadcast_to([128, NT, E])
            )

            if dbg:
                nc.sync.dma_start(
                    out=dbg_g4[:].rearrange("p (c e) -> p c e", e=E), in_=g4[:]
                )

            # ---- phase 2: batched gate replication, then uninterrupted mm2 ----
            psgt = ps_g.tile([E, TC], F32, tag="psgt")
            for t in range(NT):
                nc.tensor.transpose(
                    psgt[:, t * 128 : (t + 1) * 128], g4[:, t, :], ident[:]
                )
            gt = gp.tile([E, TC], BF16, tag="gt")
            nc.vector.tensor_copy(out=gt[:], in_=psgt[:])
            psrep = ps_g.tile([ER, TC], F32, tag="psrep")
            nc.tensor.matmul(psrep[:], repm, gt[:], start=True, stop=True)
            # per-chunk copy/mul so mm2 for chunk 0 can start before the rest;
            # midp rows 64:128 are zero padding for the K=128 bf16 mm2
            grep = gp.tile([ER, TC], F32, tag="grep")
            midp = gp.tile([128, TC], BF16, tag="midp")
            nc.gpsimd.memset(midp[ER:, :], 0.0)
            for t in range(NT):
                sl = slice(t * 128, (t + 1) * 128)
                if t % 2 == 0:
                    nc.scalar.copy(out=grep[:, sl], in_=psrep[:, sl])
                else:
                    nc.vector.tensor_copy(out=grep[:, sl], in_=psrep[:, sl])
                nc.vector.tensor_mul(midp[:ER, sl], midps[:ER, sl], grep[:, sl])
            if dbg:
                nc.sync.dma_start(out=dbg_midp[:], in_=midp[:])

            # one store per token chunk: each dma_start costs ~600ns of serial
            # SP-sequencer dispatch (DIRECT2D), so 4 stores instead of 32
            for t in range(NT):
                ob = outp.tile([128, O], BF16, tag="ob")
                for oc in range(NO):
                    pd = ps_d.tile([128, 512], F32, tag="pd")
                    nc.tensor.matmul(
                        pd[:],
                        midp[:, t * 128 : (t + 1) * 128],
                        btb[:, oc * 512 : (oc + 1) * 512],
                        start=True,
                        stop=True,
                    )
                    if (t * NO + oc) % 2 == 0:
                        nc.vector.tensor_copy(
                            out=ob[:, oc * 512 : (oc + 1) * 512], in_=pd[:]
                        )
                    else:
                        nc.scalar.copy(
                            out=ob[:, oc * 512 : (oc + 1) * 512], in_=pd[:]
                        )
                    if oc == NO // 2 - 1:
                        # store each half as soon as its evictions land
                        nc.sync.dma_start(
                            out=out[t * 128 : (t + 1) * 128, : O // 2],
                            in_=ob[:, : O // 2],
                        )
                    elif t == NT - 1 and oc == NO - 2:
                        # quarter the very last store so the post-eviction
                        # drain is halved
                        nc.sync.dma_start(
                            out=out[t * 128 : (t + 1) * 128, O // 2 : 3 * O // 4],
                            in_=ob[:, O // 2 : 3 * O // 4],
                        )
                if t == NT - 1:
                    nc.sync.dma_start(
                        out=out[t * 128 : (t + 1) * 128, 3 * O // 4 :],
                        in_=ob[:, 3 * O // 4 :],
                    )
                else:
                    nc.sync.dma_start(
                        out=out[t * 128 : (t + 1) * 128, O // 2 :],
                        in_=ob[:, O // 2 :],
                    )
    nc.finalize()
    return nc


_NC_CACHE = None


def _get_nc():
    global _NC_CACHE
    if _NC_CACHE is None:
        _NC_CACHE = _build_nc()
    return _NC_CACHE


def _prep_weights(A, B, Wr):
    W1 = np.concatenate([A.reshape(ER, F), Wr], axis=0).astype(np.float32)  # [68, F]
    # packed [128, KB*M1]: w1[p, k*M1+j] = W1[j, k*128+p]; repm appended
    w1p = np.zeros((128, KB * M1 + ER), dtype=ml_dtypes.bfloat16)
    w1p[:, : KB * M1] = (
        W1.T.reshape(KB, 128, M1).transpose(1, 0, 2).reshape(128, KB * M1)
    ).astype(ml_dtypes.bfloat16)
    for e in range(E):
        w1p[e, KB * M1 + e * R : KB * M1 + (e + 1) * R] = 1.0
    # btp [128, O]: rows 0:64 = B[e, o, r] * SCALE -> (e r) o; rows 64:128 zero
    btp = np.zeros((128, O), dtype=ml_dtypes.bfloat16)
    btp[:ER] = (B.transpose(0, 2, 1).reshape(ER, O) * SCALE).astype(ml_dtypes.bfloat16)
    return w1p, btp


def kernel(x, A, B, Wr, _trace=False, _trace_kwargs=None):
    x = np.asarray(x, dtype=np.float32)
    A = np.asarray(A, dtype=np.float32)
    B = np.asarray(B, dtype=np.float32)
    Wr = np.asarray(Wr, dtype=np.float32)

    orig_shape = x.shape
    flat = x.reshape(-1, orig_shape[-1]).astype(ml_dtypes.bfloat16)
    w1p, btp = _prep_weights(A, B, Wr)

    nc = _get_nc()
    in_maps = []
    for c in range(N_CORES):
        xc = flat[c * TC : (c + 1) * TC, :]
        # xp[kb*TC + t, p] = xc[t, kb*128 + p]
        xpc = np.ascontiguousarray(
            xc.reshape(TC, KB, 128).transpose(1, 0, 2).reshape(KB * TC, 128)
        )
        in_maps.append({"xp": xpc, "w1": w1p, "bt": btp})
    kw = {}
    if _trace:
        kw = dict(trace=True, trace_cores=[0], trace_kwargs=_trace_kwargs or {})
    res = run_bass_kernel_spmd(nc, in_maps, core_ids=list(range(N_CORES)), **kw)
    outs = [np.asarray(res.results[c]["out"], dtype=np.float32) for c in range(N_CORES)]
    full = np.concatenate(outs, axis=0).reshape(*orig_shape[:-1], O)
    if _trace:
        kernel._last_results = res
    return full


# revision 77
# speedup vs baseline: 1.1024x; 1.1024x over previous
"""MoE LoRA delta kernel for Trainium2 (Bass/Tile), 8-core SPMD.

Computation (reference):
  flat [T,F] -> logits = flat @ Wr.T [T,E]; top-2 softmax gates
  mid = flat @ A[e].T  [T,E,R];  delta = sum_e gates[:,e] * (mid[e] @ B[e].T) * SCALE

Shapes: T=4096 (2x2048), F=O=4096, E=4, R=16, SCALE=4.0.

Strategy (v2):
  - Token-shard T across 8 cores (512 tokens each); replicate tiny weights.
  - x is cast to bf16 and pre-permuted on host to xp[kb*TC+t, p] =
    x[t, kb*128+p]; the DMA XBAR transpose (dma_start(transpose=True))
    then lands x^T in SBUF as [128p, KB, TC] with fully-contiguous DRAM
    reads and zero PE/DVE transpose work.
  - Router Wr is folded into the first matmul: W1ext = [A_cat; Wr] ->
    [68, 4096]; one 32-step bf16 accumulation chain computes
    midT_ext [68, 512] (rows 0:64 = A-mid, rows 64:68 = router logits).
  - Gates: logits^T rows transposed per 128-token chunk via a tiny 4x4
    identity at partition offset 64; top-2 softmax via exp/masking on
    DVE/ACT; transpose back; replicate 4->64 rows via a 0/1 matmul.
  - mm2 per token chunk: delta [128t, 512o] = (gates*midT).T @ B_cat.T in
    fp32r (full PE rate at N=512); SCALE folded into B host-side.
  - Output staged per token chunk [128, 4096] and stored with 16KB/partition
    DMA lines.
"""

import numpy as np
import ml_dtypes

import concourse.bass as bass
import concourse.mybir as mybir
import concourse.tile as tile
from concourse import bacc, masks
from concourse.bass_utils import run_bass_kernel_spmd

F32 = mybir.dt.float32
F32R = mybir.dt.float32r
BF16 = mybir.dt.bfloat16

N_CORES = 8
T_FULL = 4096          # 2*2048 tokens
TC = T_FULL // N_CORES  # 512 tokens per core
F = 4096
O = 4096
E = 4
R = 16
ER = E * R             # 64
M1 = ER + E            # 68 (A rows + router rows)
KB = F // 128          # 32 f-blocks
NT = TC // 128         # 4 token chunks per core
NO = O // 512          # 8 output column chunks
NG = 4                 # x^T DMA-transpose groups
G = KB // NG           # k-blocks per group
SCALE = 16.0 / np.sqrt(16.0)  # 4.0


def _build_nc(dbg=False):
    nc = bacc.Bacc(
        "TRN2", debug=False, target_bir_lowering=False, enable_partition_id=False
    )

    # xp packed+transposed host-side: xp[kb*TC + t, p] = bf16(x)[t, kb*128+p]
    xp = nc.dram_tensor("xp", [KB * TC, 128], BF16, kind="ExternalInput")
    # w1 packed: [128, KB*M1 (+ER repm tail)]; w1[p, k*M1 + j] = W1ext[j, k*128 + p]
    # where W1ext = concat([A_cat (64 rows), Wr (4 rows)])  (bf16);
    # the trailing ER columns of rows 0:4 hold the 4->64 replication matrix
    w1 = nc.dram_tensor("w1", [128, KB * M1 + ER], BF16, kind="ExternalInput")
    # bt: [128, O]; rows 0:64 hold bt[e*R+r, o] = B[e, o, r] * SCALE, rows
    # 64:128 are zero padding: a hot bf16 K=128 matmul sustains a ~260ns
    # cadence (incl. weight reload) vs f32r-K=64's bandwidth-capped ~427ns
    bt = nc.dram_tensor("bt", [128, O], BF16, kind="ExternalInput")
    # bf16 output (host upconverts to f32): halves the output-DMA window,
    # which otherwise gates the entire mm2 phase
    out = nc.dram_tensor("out", [TC, O], BF16, kind="ExternalOutput")
    if dbg:
        dbg_mid = nc.dram_tensor("dbg_mid", [M1, TC], F32, kind="ExternalOutput")
        dbg_lg4 = nc.dram_tensor("dbg_lg4", [128, NT * E], F32, kind="ExternalOutput")
        dbg_g4 = nc.dram_tensor("dbg_g4", [128, NT * E], F32, kind="ExternalOutput")
        dbg_midp = nc.dram_tensor("dbg_midp", [ER, TC], F32R, kind="ExternalOutput")

    with tile.TileContext(nc) as tc:
        with (
            tc.tile_pool(name="consts", bufs=1) as consts,
            tc.tile_pool(name="xt", bufs=1) as xt_pool,
            tc.tile_pool(name="gates", bufs=2) as gp,
            tc.tile_pool(name="outp", bufs=4) as outp,
            tc.tile_pool(name="ps_mid", bufs=1, space="PSUM") as ps_mid,
            tc.tile_pool(name="ps_lg", bufs=1, space="PSUM") as ps_lg,
            tc.tile_pool(name="ps_g", bufs=1, space="PSUM") as ps_g,
            tc.tile_pool(name="ps_d", bufs=4, space="PSUM") as ps_d,
        ):
            # ---- constants / weights ----
            ident = consts.tile([128, 128], F32)
            masks.make_identity(nc, ident[:])
            # tiny 4x4 identity at partition offset 64 (for logit transposes:
            # lhsT/rhs must share a base partition, and logits live on 64:68)
            idq = consts.tile([128, E], F32)
            masks.make_identity(nc, idq[ER:M1, :])

            # ---- phase 1: XBAR-transpose x into SBUF, mm1 chain ----
            # The XBAR is a single serial unit and DMA traffic executes in
            # queue order, so the first transpose goes out immediately and the
            # small weight loads are interleaved between transposes. Group
            # sizes are uneven (10/10/10/2) so the last group's mm1 tail after
            # the final transpose is short.
            GS = [10, 10, 10, 2]
            GOFF = [0, 10, 20, 30]
            WARMS = [4, 4, 4]  # per-boundary warm counts
            # All DMA traffic is one serial pipe (the XBAR does not overlap
            # regular DMA), so order it by when each byte is first needed:
            # only the first group's w1 slice precedes XBAR g0; the rest of
            # the weights slot in between transposes. mm1 is PE-serial-bound,
            # and an earlier g0 directly advances its start.
            # Each XBAR<->regular-DMA mode transition stalls the sequencer
            # ~3us, so the XBAR transposes run as one consecutive block: w1
            # (needed by mm1 k=0) loads before it, bt (needed only by mm2,
            # ~30us later) after it.
            w1s = consts.tile([128, KB * M1 + ER], BF16)
            repm = w1s[0:E, KB * M1 : KB * M1 + ER]
            btb = consts.tile([128, O], BF16)
            nc.sync.dma_start(out=w1s[:], in_=w1[:])

            xts = []
            for g in range(NG):
                xT = xt_pool.tile([128, GS[g], TC], BF16, tag=f"xT{g}")
                nc.sync.dma_start(
                    out=xT[:],
                    in_=xp[GOFF[g] * TC : (GOFF[g] + GS[g]) * TC, :],
                    transpose=True,
                )
                xts.append(xT)

            nc.sync.dma_start(out=btb[:], in_=bt[:])

            def warm(n, g):
                # keep the PE's p-state hot across stalls: idle gaps drop it
                # to 1.2 GHz with a 3us re-ramp; a hot bf16 K=128 matmul
                # sustains ~260ns cadence vs ~430ns cold
                for _ in range(n):
                    pdw = ps_d.tile([128, 512], F32, tag="pd")
                    nc.tensor.matmul(
                        pdw[:], w1s[:, 0:128], xts[g][:, 0, :],
                        start=True, stop=True,
                    )

            midps = ps_mid.tile([M1, TC], F32)  # rows 0:64 mid, 64:68 logits
            for k in range(KB):
                g = next(i for i in range(NG) if GOFF[i] <= k < GOFF[i] + GS[i])
                nc.tensor.matmul(
                    midps[:],
                    w1s[:, k * M1 : (k + 1) * M1],
                    xts[g][:, k - GOFF[g], :],
                    start=(k == 0),
                    stop=(k == KB - 1),
                )
                if k == GOFF[g] + GS[g] - 1 and g < NG - 1:
                    warm(WARMS[g], g)  # bridge the wait for the next group
            warm(2, NG - 1)  # bridge the logits-eviction wait

            # ---- gates ----
            # logits^T [4, 512] live on partitions 64:68; evict lane-aligned
            lgT = gp.tile([128, TC], F32, tag="lgT")
            nc.vector.tensor_copy(out=lgT[ER:M1, :], in_=midps[ER:M1, :])
            # transpose to [t, e] layout per 128-token chunk
            lgps = ps_lg.tile([128, NT, E], F32)
            for t in range(NT):
                nc.tensor.matmul(
                    lgps[:, t, :],
                    lgT[ER:M1, t * 128 : (t + 1) * 128],
                    idq[ER:M1, :],
                    is_transpose=True,
                )
            # keep the PE continuously busy through the DVE softmax window:
            # an idle gap drops the PE to the 1.2 GHz p-state and re-ramping
            # to 2.4 GHz takes 3us of continuous execution, which would slow
            # every psgt/repmat/mm2 matmul afterwards to ~2x
            for i in range(10):
                pdw = ps_d.tile([128, 512], F32, tag="pd")
                nc.tensor.matmul(
                    pdw[:], w1s[:, 0:128], xts[NG - 1][:, 0, :],
                    start=True, stop=True,
                )

            lg4 = lgps  # softmax reads the PSUM logits directly
            if dbg:
                dmid = gp.tile([M1, TC], F32, tag="dmid")
                nc.vector.tensor_copy(out=dmid[:], in_=midps[:])
                nc.sync.dma_start(out=dbg_mid[:], in_=dmid[:])
                dlg = gp.tile([128, NT, E], F32, tag="dlg")
                nc.vector.tensor_copy(out=dlg[:], in_=lgps[:])
                nc.sync.dma_start(
                    out=dbg_lg4[:].rearrange("p (c e) -> p c e", e=E), in_=dlg[:]
                )

            # batched top-2 softmax over all chunks at once ([128, NT, E]);
            # no max-subtraction (logits are small, exp is safe in f32, and
            # exact equality with the running max identifies top-1/top-2)
            g4 = gp.tile([128, NT, E], F32, tag="g4")
            et = gp.tile([128, NT, E], F32, tag="et")
            nc.scalar.activation(et[:], lg4[:], mybir.ActivationFunctionType.Exp)
            mx4 = gp.tile([128, NT], F32, tag="mx4")
            nc.vector.reduce_max(mx4[:], et[:], axis=mybir.AxisListType.X)
            m1 = gp.tile([128, NT, E], F32, tag="m1")
            nc.vector.tensor_tensor(
                out=m1[:], in0=et[:],
                in1=mx4[:, :, None].broadcast_to([128, NT, E]),
                op=mybir.AluOpType.is_lt,
            )
            et2 = gp.tile([128, NT, E], F32, tag="et2")
            nc.vector.tensor_mul(et2[:], et[:], m1[:])  # et with top-1 zeroed
            mx2 = gp.tile([128, NT], F32, tag="mx2")
            nc.vector.reduce_max(mx2[:], et2[:], axis=mybir.AxisListType.X)
            m2 = gp.tile([128, NT, E], F32, tag="m2")
            nc.vector.tensor_tensor(
                out=m2[:], in0=et[:],
                in1=mx2[:, :, None].broadcast_to([128, NT, E]),
                op=mybir.AluOpType.is_ge,
            )
            em = gp.tile([128, NT, E], F32, tag="em")
            nc.vector.tensor_mul(em[:], et[:], m2[:])
            z = gp.tile([128, NT], F32, tag="z")
            nc.vector.reduce_sum(z[:], em[:], axis=mybir.AxisListType.X)
            rz = gp.tile([128, NT], F32, tag="rz")
            nc.vector.reciprocal(rz[:], z[:])
            nc.vector.tensor_mul(
                g4[:], em[:], rz[:, :, None].broadcast_to([128, NT, E])
            )

            if dbg:
                nc.sync.dma_start(
                    out=dbg_g4[:].rearrange("p (c e) -> p c e", e=E), in_=g4[:]
                )

            # ---- phase 2: batched gate replication, then uninterrupted mm2 ----
            psgt = ps_g.tile([E, TC], F32, tag="psgt")
            for t in range(NT):
                nc.tensor.transpose(
                    psgt[:, t * 128 : (t + 1) * 128], g4[:, t, :], ident[:]
                )
            gt = gp.tile([E, TC], BF16, tag="gt")
            nc.vector.tensor_copy(out=gt[:], in_=psgt[:])
            psrep = ps_g.tile([ER, TC], F32, tag="psrep")
            nc.tensor.matmul(psrep[:], repm, gt[:], start=True, stop=True)
            # per-chunk copy/mul so mm2 for chunk 0 can start before the rest;
            # midp rows 64:128 are zero padding for the K=128 bf16 mm2
            grep = gp.tile([ER, TC], F32, tag="grep")
            midp = gp.tile([128, TC], BF16, tag="midp")
            nc.gpsimd.memset(midp[ER:, :], 0.0)
            for t in range(NT):
                sl = slice(t * 128, (t + 1) * 128)
                if t % 2 == 0:
                    nc.scalar.copy(out=grep[:, sl], in_=psrep[:, sl])
                else:
                    nc.vector.tensor_copy(out=grep[:, sl], in_=psrep[:, sl])
                nc.vector.tensor_mul(midp[:ER, sl], midps[:ER, sl], grep[:, sl])
            if dbg:
                nc.sync.dma_start(out=dbg_midp[:], in_=midp[:])

            # one store per token chunk: each dma_start costs ~600ns of serial
            # SP-sequencer dispatch (DIRECT2D), so 4 stores instead of 32
            for t in range(NT):
                ob = outp.tile([128, O], BF16, tag="ob")
                for oc in range(NO):
                    pd = ps_d.tile([128, 512], F32, tag="pd")
                    nc.tensor.matmul(
                        pd[:],
                        midp[:, t * 128 : (t + 1) * 128],
                        btb[:, oc * 512 : (oc + 1) * 512],
                        start=True,
                        stop=True,
                    )
                    if (t * NO + oc) % 2 == 0:
                        nc.vector.tensor_copy(
                            out=ob[:, oc * 512 : (oc + 1) * 512], in_=pd[:]
                        )
                    else:
                        nc.scalar.copy(
                            out=ob[:, oc * 512 : (oc + 1) * 512], in_=pd[:]
                        )
                    if oc == NO // 2 - 1:
                        # store each half as soon as its evictions land
                        nc.sync.dma_start(
                            out=out[t * 128 : (t + 1) * 128, : O // 2],
                            in_=ob[:, : O // 2],
                        )
                nc.sync.dma_start(
                    out=out[t * 128 : (t + 1) * 128, O // 2 :],
                    in_=ob[:, O // 2 :],
                )
    nc.finalize()
    return nc


_NC_CACHE = None


def _get_nc():
    global _NC_CACHE
    if _NC_CACHE is None:
        _NC_CACHE = _build_nc()
    return _NC_CACHE


def _prep_weights(A, B, Wr):
    W1 = np.concatenate([A.reshape(ER, F), Wr], axis=0).astype(np.float32)  # [68, F]
    # packed [128, KB*M1]: w1[p, k*M1+j] = W1[j, k*128+p]; repm appended
    w1p = np.zeros((128, KB * M1 + ER), dtype=ml_dtypes.bfloat16)
    w1p[:, : KB * M1] = (
        W1.T.reshape(KB, 128, M1).transpose(1, 0, 2).reshape(128, KB * M1)
    ).astype(ml_dtypes.bfloat16)
    for e in range(E):
        w1p[e, KB * M1 + e * R : KB * M1 + (e + 1) * R] = 1.0
    # btp [128, O]: rows 0:64 = B[e, o, r] * SCALE -> (e r) o; rows 64:128 zero
    btp = np.zeros((128, O), dtype=ml_dtypes.bfloat16)
    btp[:ER] = (B.transpose(0, 2, 1).reshape(ER, O) * SCALE).astype(ml_dtypes.bfloat16)
    return w1p, btp


def kernel(x, A, B, Wr, _trace=False, _trace_kwargs=None):
    x = np.asarray(x, dtype=np.float32)
    A = np.asarray(A, dtype=np.float32)
    B = np.asarray(B, dtype=np.float32)
    Wr = np.asarray(Wr, dtype=np.float32)

    orig_shape = x.shape
    flat = x.reshape(-1, orig_shape[-1]).astype(ml_dtypes.bfloat16)
    w1p, btp = _prep_weights(A, B, Wr)

    nc = _get_nc()
    in_maps = []
    for c in range(N_CORES):
        xc = flat[c * TC : (c + 1) * TC, :]
        # xp[kb*TC + t, p] = xc[t, kb*128 + p]
        xpc = np.ascontiguousarray(
            xc.reshape(TC, KB, 128).transpose(1, 0, 2).reshape(KB * TC, 128)
        )
        in_maps.append({"xp": xpc, "w1": w1p, "bt": btp})
    kw = {}
    if _trace:
        kw = dict(trace=True, trace_cores=[0], trace_kwargs=_trace_kwargs or {})
    res = run_bass_kernel_spmd(nc, in_maps, core_ids=list(range(N_CORES)), **kw)
    outs = [np.asarray(res.results[c]["out"], dtype=np.float32) for c in range(N_CORES)]
    full = np.concatenate(outs, axis=0).reshape(*orig_shape[:-1], O)
    if _trace:
        kernel._last_results = res
    return full
